# revision 2
# baseline (speedup 1.0000x reference)
"""ConvNeXt block kernel for Trainium2 — fp8 DoubleRow rewrite.

Reference semantics (per image):
  y = x + gamma * ( GELU( LN(dwconv7x7(x) + dw_b) @ w1 + b1 ) @ w2 + b2 )
with LN over channels, exact (erf) GELU, NCHW in/out.

Distribution: batch 16 -> 2 images per core across 8 cores. No collectives.

Everything heavy runs on the tensor engine in fp8 DoubleRow mode (2 fp8
k-blocks per pass, 0.5 cycles/row):
 - depthwise 7x7 conv: channels on partitions, diagonal-weight matmuls over a
   zero-padded 62x62 fp8 image; 49 taps + 1 zero tap = 25 DR pairs per
   448-pixel chunk, moving APs [128, 2, 8, 56] built directly with pair
   stride = tap-offset delta (weights pre-scaled x256, undone in the
   psum->acc bias activation).
 - MLP d->4d->4d->d: fp8 DR over k-blocks; the LN is folded in: xq holds
   y*rstd in blocks 0-2 and a 4th block whose row0 = mu*rstd (mean
   correction via s1n row) and row1 = 1.0 (carries b1), both x16; w2 carries
   x32, b2 rides a k=1 matmul of 32*b2 against a const-ones row; gamma/32 and
   the fp32 residual are applied by one DVE scalar_tensor_tensor straight
   from PSUM.
LN stats: ones-matmuls for mean/meansq (bf16), variance -> DRAM-bounced
[56,8] transposed layout, magic-constant Newton rsqrt on DVE (2 iters),
rstd broadcast to 128 partitions by DMA. Squares batched on ACT.
"""

import sys

sys.path.insert(0, "/opt/trn_rl_repo")

import numpy as np
import ml_dtypes

import bass_rust
import concourse.bass as bass
import concourse.mybir as mybir
import concourse.tile as tile
from concourse.bass_utils import run_bass_kernel_spmd

F32 = mybir.dt.float32
BF16 = mybir.dt.bfloat16
FP8 = mybir.dt.float8e4
I32 = mybir.dt.int32
AF = mybir.ActivationFunctionType
ALU = mybir.AluOpType
DRM = mybir.MatmulPerfMode.DoubleRow

N_CORES = 8
IMGS_PER_CORE = 2
C = 384
CB = 3          # channel blocks of 128
H = W = 56
PIX = H * W     # 3136
RPAD = 62       # 3 + 56 + 3 rows
WPAD = 62       # 3 + 56 + 3 cols
CHUNK = 448     # pixels per chunk (8 rows)
NCHUNK = 7
FD = 1536       # hidden dim
NFC = 12        # hidden blocks of 128
EPS = 1e-6

WSCALE = 256.0   # conv tap weights
W1SCALE = 16.0   # w1 / s1n / b1 rows
W2SCALE = 32.0   # w2 / b2 row

MAGIC = 0x5F3759DF

_WAITSPLIT_N = [0]


def _split_waits(nc, max_waits=1):
    """This walrus build rejects instructions with more than one sync-wait
    command; hoist excess waits onto dedicated NoOps on the same engine."""
    for fn in nc.m.functions:
        for bb in fn.blocks:
            insts = bb.instructions
            idx = 0
            while idx < len(insts):
                ins = insts[idx]
                si = ins.sync_info
                if si is not None and len(si.on_wait) > max_waits:
                    waits = list(si.on_wait)
                    extra, keep = waits[:-max_waits], waits[-max_waits:]
                    nops = []
                    for w in extra:
                        _WAITSPLIT_N[0] += 1
                        nops.append(
                            mybir.InstNoOp(
                                name=f"I-wsplit-{_WAITSPLIT_N[0]}",
                                engine=ins.engine,
                                ins=[],
                                outs=[],
                                sync_info=bass_rust.SyncInfo(
                                    on_wait=[w], on_update=[]
                                ),
                            )
                        )
                    ins.sync_info = bass_rust.SyncInfo(
                        on_wait=keep, on_update=list(si.on_update)
                    )
                    insts[idx:idx] = nops
                    idx += len(nops)
                idx += 1


def _sv(base_ap, extra_off, dims):
    """Strided AP view over base_ap's tensor: partition dim kept, free dims
    given as (size, stride) pairs, offset in elements."""
    pairs = [list(base_ap.ap[0])] + [[s, n] for n, s in dims]
    return bass.AP(base_ap.tensor, base_ap.offset + extra_off,
                   bass_rust.VecI64Pair(pairs))


def _de_off(de, hc0):
    d, e = de
    return (hc0 + d + 3) * WPAD + (e + 3)


def _conv_pairs():
    """49 taps + 1 zero slot as 25 DoubleRow pairs. The pair-dim stride of
    the moving AP must be EVEN (odd fp8 strides hang the PE): row pairs
    (d,e)&(d+1,e) have delta 62; row-3 pairs (3,e)&(3,e+2) have delta 2;
    the leftover (3,2) rides second in a pair whose first slot is a zero
    tap reading 2 elements earlier."""
    pairs = []
    for d in (-3, -1, 1):
        for e in range(-3, 4):
            pairs.append(((d, e), (d + 1, e)))
    for e1, e2 in ((-3, -1), (-2, 0), (1, 3)):
        pairs.append(((3, e1), (3, e2)))
    pairs.append((None, (3, 2)))
    return pairs


PAIRS = _conv_pairs()


def _build_nc():
    nc = bass.Bass(trn_type="TRN2", target_bir_lowering=False, debug=False)

    xs = nc.dram_tensor("xs", [IMGS_PER_CORE, C, H, W], F32, kind="ExternalInput")
    dg = nc.dram_tensor("dg", [128, CB, 50, 128], FP8, kind="ExternalInput")
    dwb = nc.dram_tensor("dwb", [C], F32, kind="ExternalInput")
    w1q = nc.dram_tensor("w1q", [128, 4, FD], FP8, kind="ExternalInput")
    w2q = nc.dram_tensor("w2q", [128, NFC, C], FP8, kind="ExternalInput")
    b1p = nc.dram_tensor("b1p", [FD], F32, kind="ExternalInput")
    b2r = nc.dram_tensor("b2r", [1, C], FP8, kind="ExternalInput")
    gsc = nc.dram_tensor("gsc", [C], F32, kind="ExternalInput")
    ys = nc.dram_tensor("ys", [IMGS_PER_CORE, C, H, W], F32, kind="ExternalOutput")
    vscratch = nc.dram_tensor("vscratch", [IMGS_PER_CORE, PIX], F32, kind="Internal")
    rscratch = nc.dram_tensor("rscratch", [IMGS_PER_CORE, PIX], F32, kind="Internal")

    xs3 = xs.ap().rearrange("i c h w -> i c (h w)")
    ys3 = ys.ap().rearrange("i c h w -> i c (h w)")

    with tile.TileContext(nc) as tc:
        with (
            tc.tile_pool(name="const", bufs=1) as constp,
            tc.tile_pool(name="xstage", bufs=2) as xstagep,
            tc.tile_pool(name="xpad", bufs=1) as xpadp,
            tc.tile_pool(name="acc", bufs=1) as accp,
            tc.tile_pool(name="ysq", bufs=2) as ysqp,
            tc.tile_pool(name="xq", bufs=2) as xqp,
            tc.tile_pool(name="hq", bufs=2) as hqp,
            tc.tile_pool(name="rb", bufs=2) as rbp,
            tc.tile_pool(name="small", bufs=3) as smallp,
            tc.tile_pool(name="out", bufs=3) as outp,
            tc.tile_pool(name="psconv", bufs=2, space="PSUM") as psconvp,
            tc.tile_pool(name="ps1", bufs=2, space="PSUM") as ps1p,
            tc.tile_pool(name="ps2", bufs=2, space="PSUM") as ps2p,
            tc.tile_pool(name="psstat", bufs=2, space="PSUM") as psstatp,
        ):
            # ---- static weights / constants ----
            dg_sb = constp.tile([128, CB, 50, 128], FP8)
            nc.sync.dma_start(dg_sb[:], dg.ap())
            dwb_sb = constp.tile([128, CB], F32)
            nc.sync.dma_start(dwb_sb[:], dwb.ap().rearrange("(cb p) -> p cb", p=128))
            w1_sb = constp.tile([128, 4, FD], FP8)
            nc.sync.dma_start(w1_sb[:], w1q.ap())
            w2_sb = constp.tile([128, NFC, C], FP8)
            nc.sync.dma_start(w2_sb[:], w2q.ap())
            b1_sb = constp.tile([128, NFC], F32)
            nc.sync.dma_start(b1_sb[:], b1p.ap().rearrange("(fc p) -> p fc", p=128))
            b2_sb = constp.tile([1, C], FP8)
            nc.sync.dma_start(b2_sb[:], b2r.ap())
            gsc_sb = constp.tile([128, CB], F32)
            nc.sync.dma_start(gsc_sb[:], gsc.ap().rearrange("(cb p) -> p cb", p=128))
            ones_bf = constp.tile([128, 1], BF16)
            nc.gpsimd.memset(ones_bf[:], 1.0)
            ones8 = constp.tile([1, CHUNK], FP8)
            nc.gpsimd.memset(ones8[:], 1.0)
            s256 = constp.tile([128, 1], F32)
            nc.gpsimd.memset(s256[:], 1.0 / WSCALE)
            s16 = constp.tile([128, 1], F32)
            nc.gpsimd.memset(s16[:], 1.0 / W1SCALE)

            # padded fp8 image tiles, one per (img, cb); pads zeroed once
            xpads = {}
            for img in range(IMGS_PER_CORE):
                for cb in range(CB):
                    t = xpadp.tile([128, RPAD * WPAD], FP8, tag=f"xpad{img}{cb}", name=f"xpad{img}{cb}")
                    nc.gpsimd.memset(t[:], 0.0)
                    xpads[(img, cb)] = t

            accs = {}
            for img in range(IMGS_PER_CORE):
                acc_t = accp.tile([128, CB, PIX], BF16, tag=f"acc{img}", name=f"acc{img}")
                accs[img] = acc_t

            # ---------------- emission helpers ----------------
            def emit_input_prep(img, cb):
                cs = slice(cb * 128, (cb + 1) * 128)
                xstg = xstagep.tile([128, PIX], BF16, tag="xstg")
                nc.gpsimd.dma_start(xstg[:], xs3[img, cs])
                xp = xpads[(img, cb)]
                xp3 = xp.rearrange("p (r w) -> p r w", w=WPAD)
                nc.gpsimd.tensor_copy(
                    xp3[:, 3:59, 3:59],
                    xstg.rearrange("p (h w) -> p h w", w=W),
                )

            def emit_conv_chunk(img, ch):
                hc0 = ch * 8
                for cb in range(CB):
                    xp = xpads[(img, cb)]
                    pst = psconvp.tile([128, CHUNK], F32, tag="psconv")
                    for p, (t1, t2) in enumerate(PAIRS):
                        if t1 is None:
                            base = _de_off(t2, hc0) - 2
                            delta = 2
                        else:
                            base = _de_off(t1, hc0)
                            delta = _de_off(t2, hc0) - base
                        nc.tensor.matmul(
                            pst[:], dg_sb[:, cb, 2 * p : 2 * p + 2, :],
                            _sv(xp[:], base, [(2, delta), (8, WPAD), (56, 1)]),
                            start=(p == 0), stop=(p == len(PAIRS) - 1),
                            perf_mode=DRM,
                        )
                    nc.scalar.activation(
                        accs[img][:, cb, hc0 * W : hc0 * W + CHUNK], pst[:],
                        AF.Identity, bias=dwb_sb[:, cb : cb + 1],
                        scale=s256[:, 0:1],
                    )

            def emit_stats(img, ch):
                """LN stats + rstd chain + xq for one chunk. Returns xq tile."""
                sl = slice(ch * CHUNK, (ch + 1) * CHUNK)
                acc = accs[img]
                pmu = psstatp.tile([1, CHUNK], F32, tag="ps_stat")
                for cb in range(CB):
                    nc.tensor.matmul(
                        pmu[:], ones_bf[:, 0:1], acc[:, cb, sl],
                        start=(cb == 0), stop=(cb == CB - 1),
                    )
                mu32 = smallp.tile([1, CHUNK], F32, tag="mu32")
                nc.vector.tensor_scalar_mul(mu32[:], pmu[:], 1.0 / C)
                ysq = ysqp.tile([128, CB, CHUNK], BF16, tag="ysq")
                nc.scalar.activation(ysq[:], acc[:, :, sl], AF.Square)
                pmsq = psstatp.tile([1, CHUNK], F32, tag="ps_stat")
                for cb in range(CB):
                    nc.tensor.matmul(
                        pmsq[:], ones_bf[:, 0:1], ysq[:, cb, :],
                        start=(cb == 0), stop=(cb == CB - 1),
                    )
                tq = smallp.tile([1, CHUNK], F32, tag="tq")
                nc.vector.tensor_mul(tq[:], mu32[:], mu32[:])
                vchunk = smallp.tile([1, CHUNK], F32, tag="vchunk")
                nc.vector.scalar_tensor_tensor(
                    out=vchunk[:], in0=pmsq[:], scalar=1.0 / C, in1=tq[:],
                    op0=ALU.mult, op1=ALU.subtract,
                )
                nc.sync.dma_start(vscratch.ap()[img : img + 1, sl], vchunk[0:1, :])

                # Newton rsqrt in [56,8] transposed layout (DRAM bounce)
                vpf = smallp.tile([56, 8], F32, tag="vpf")
                nc.sync.dma_start(
                    vpf[:], vscratch.ap()[img, sl].rearrange("(p f) -> p f", p=56)
                )
                v_eps = smallp.tile([56, 8], F32, tag="veps")
                nc.vector.tensor_scalar_add(v_eps[:], vpf[:], EPS)
                yr = smallp.tile([56, 8], F32, tag="yr")
                ti = smallp.tile([56, 8], I32, tag="ti")
                nc.vector.tensor_scalar(
                    ti[:], v_eps[:].bitcast(I32), 1, None, ALU.logical_shift_right
                )
                nc.vector.tensor_scalar(ti[:], ti[:], -1, None, ALU.bitwise_xor)
                nc.vector.tensor_scalar(yr[:].bitcast(I32), ti[:], MAGIC + 1, None, ALU.add)
                rr = smallp.tile([56, 8], F32, tag="rr")
                for _ in range(2):
                    nc.vector.tensor_mul(rr[:], yr[:], yr[:])
                    nc.vector.tensor_mul(rr[:], rr[:], v_eps[:])
                    nc.vector.tensor_scalar(rr[:], rr[:], -0.5, 1.5, ALU.mult, ALU.add)
                    nc.vector.tensor_mul(yr[:], yr[:], rr[:])
                nc.sync.dma_start(
                    rscratch.ap()[img, sl].rearrange("(p f) -> p f", p=56), yr[:]
                )
                rb = rbp.tile([128, CHUNK], BF16, tag="rb")
                nc.gpsimd.dma_start(rb[:], rscratch.ap()[img, sl].partition_broadcast(128))

                xq = xqp.tile([128, 4, CHUNK], FP8, tag="xq")
                nc.gpsimd.memset(xq[:, 3, :], 0.0)
                nc.vector.tensor_mul(xq[0:1, 3, :], mu32[:], rb[0:1, :])
                for cb in range(CB):
                    nc.vector.tensor_mul(xq[:, cb, :], acc[:, cb, sl], rb[:])
                return xq

            def emit_p1(img, ch, xq):
                hq = hqp.tile([128, NFC, CHUNK], FP8, tag="hq")
                for fc in range(NFC):
                    fsl = slice(fc * 128, (fc + 1) * 128)
                    p1 = ps1p.tile([128, CHUNK], F32, tag="p1")
                    nc.tensor.matmul(p1[:], w1_sb[:, 0:2, fsl], xq[:, 0:2, :],
                                     start=True, stop=False, perf_mode=DRM)
                    nc.tensor.matmul(p1[:], w1_sb[:, 2:4, fsl], xq[:, 2:4, :],
                                     start=False, stop=True, perf_mode=DRM)
                    nc.scalar.activation(hq[:, fc, :], p1[:], AF.Gelu,
                                         bias=b1_sb[:, fc : fc + 1],
                                         scale=s16[:, 0:1])
                return hq

            def emit_p2(img, ch, hq):
                sl = slice(ch * CHUNK, (ch + 1) * CHUNK)
                for cb in range(CB):
                    cs = slice(cb * 128, (cb + 1) * 128)
                    p2 = ps2p.tile([128, CHUNK], F32, tag="p2")
                    for j in range(NFC // 2):
                        nc.tensor.matmul(
                            p2[:], w2_sb[:, 2 * j : 2 * j + 2, cs],
                            hq[:, 2 * j : 2 * j + 2, :],
                            start=(j == 0), stop=False, perf_mode=DRM,
                        )
                    nc.tensor.matmul(p2[:], b2_sb[:, cs], ones8[:, :],
                                     start=False, stop=True)
                    xres = outp.tile([128, CHUNK], F32, tag="xres")
                    nc.sync.dma_start(xres[:], xs3[img, cs, sl])
                    ysb = outp.tile([128, CHUNK], F32, tag="ysb")
                    nc.vector.scalar_tensor_tensor(
                        out=ysb[:], in0=p2[:], scalar=gsc_sb[:, cb : cb + 1],
                        in1=xres[:], op0=ALU.mult, op1=ALU.add,
                    )
                    nc.sync.dma_start(ys3[img, cs, sl], ysb[:])

            # ---------------- software-pipelined emission ----------------
            # PE stream per stage i: stats(i) | conv(i+2) | p1(i) | p2(i-1).
            # conv between stats and p1 hides the rstd chain latency; p2 runs
            # one stage late so ACT's 12 GELUs have a full stage to complete.
            chunks = [(img, ch) for img in range(IMGS_PER_CORE)
                      for ch in range(NCHUNK)]
            for cb in range(CB):
                emit_input_prep(0, cb)
            emit_conv_chunk(*chunks[0])
            emit_conv_chunk(*chunks[1])
            conv_next = 2
            pending = None  # (img, ch, hq) awaiting p2
            for i, (img, ch) in enumerate(chunks):
                if 2 <= i <= 4:
                    emit_input_prep(1, i - 2)
                xq = emit_stats(img, ch)
                if conv_next < len(chunks):
                    emit_conv_chunk(*chunks[conv_next])
                    conv_next += 1
                hq = emit_p1(img, ch, xq)
                if pending is not None:
                    emit_p2(*pending)
                pending = (img, ch, hq)
            emit_p2(*pending)

    _split_waits(nc)
    return nc


_NC_CACHE = None


def _host_fold(inputs):
    dw_w = np.asarray(inputs["dw_w"], dtype=np.float32)
    dw_b = np.asarray(inputs["dw_b"], dtype=np.float32)
    ln_w = np.asarray(inputs["ln_w"], dtype=np.float32)
    ln_b = np.asarray(inputs["ln_b"], dtype=np.float32)
    w1 = np.asarray(inputs["w1"], dtype=np.float32)
    b1 = np.asarray(inputs["b1"], dtype=np.float32)
    w2 = np.asarray(inputs["w2"], dtype=np.float32)
    b2 = np.asarray(inputs["b2"], dtype=np.float32)
    gamma = np.asarray(inputs["gamma"], dtype=np.float32)

    f8 = ml_dtypes.float8_e4m3

    # conv taps -> diag fp8 [128, cb, 50, 128]; slots (2p, 2p+1) hold the
    # two taps of PAIRS[p] (zeros for the dummy slot)
    wt = dw_w[:, :, 0, :] * WSCALE  # [7, 7, C]
    dgq = np.zeros((128, CB, 50, 128), dtype=f8)
    idx = np.arange(128)
    for cb in range(CB):
        for p, pair in enumerate(PAIRS):
            for s, de in enumerate(pair):
                if de is None:
                    continue
                d, e = de
                wq = (wt[d + 3, e + 3, cb * 128 : (cb + 1) * 128]).astype(f8)
                dgq[idx, cb, 2 * p + s, idx] = wq

    w1p = ln_w[:, None] * w1                       # LN scale into w1
    b1p = b1 + ln_b @ w1                           # LN shift into b1
    s1n = -w1p.sum(axis=0)                         # mean-correction row
    w1qa = np.zeros((128, 4, FD), dtype=f8)
    for cb in range(CB):
        w1qa[:, cb, :] = (w1p[cb * 128 : (cb + 1) * 128] * W1SCALE).astype(f8)
    w1qa[0, 3, :] = (s1n * W1SCALE).astype(f8)

    w2qa = np.zeros((128, NFC, C), dtype=f8)
    for j in range(NFC):
        w2qa[:, j, :] = (w2[j * 128 : (j + 1) * 128] * W2SCALE).astype(f8)
    b2q = (b2 * W2SCALE).astype(f8).reshape(1, C)
    gscv = (gamma / W2SCALE).astype(np.float32)

    return {
        "dg": dgq,
        "dwb": dw_b.astype(np.float32),
        "b1p": b1p.astype(np.float32),
        "w1q": w1qa,
        "w2q": w2qa,
        "b2r": b2q,
        "gsc": gscv,
    }


def make_in_maps(inputs):
    x = np.asarray(inputs["x"], dtype=np.float32)
    common = _host_fold(inputs)
    in_maps = []
    for k in range(N_CORES):
        m = dict(common)
        m["xs"] = np.ascontiguousarray(x[k * IMGS_PER_CORE : (k + 1) * IMGS_PER_CORE])
        in_maps.append(m)
    return in_maps


def kernel(**inputs):
    global _NC_CACHE
    in_maps = make_in_maps(inputs)
    if _NC_CACHE is None:
        _NC_CACHE = _build_nc()
    res = run_bass_kernel_spmd(_NC_CACHE, in_maps, core_ids=list(range(N_CORES)))
    out = np.concatenate([res.results[k]["ys"] for k in range(N_CORES)], axis=0)
    return out.astype(np.float32)


if __name__ == "__main__":
    rng = np.random.default_rng(0)
    ins = {
        "x": rng.standard_normal((16, C, H, W), dtype=np.float32),
        "dw_w": 0.02 * rng.standard_normal((7, 7, 1, C), dtype=np.float32),
        "dw_b": 0.02 * rng.standard_normal((C,), dtype=np.float32),
        "ln_w": np.ones(C, np.float32),
        "ln_b": np.zeros(C, np.float32),
        "w1": (C**-0.5) * rng.standard_normal((C, FD), dtype=np.float32),
        "b1": 0.02 * rng.standard_normal((FD,), dtype=np.float32),
        "w2": ((4 * C) ** -0.5) * rng.standard_normal((FD, C), dtype=np.float32),
        "b2": 0.02 * rng.standard_normal((C,), dtype=np.float32),
        "gamma": np.full((C,), 1e-6, np.float32),
    }
    out = kernel(**ins)
    print("out", out.shape, out.dtype, np.abs(out).mean())


# revision 3
# speedup vs baseline: 1.3265x; 1.3265x over previous
"""ConvNeXt block kernel for Trainium2 — fp8 DoubleRow rewrite.

Reference semantics (per image):
  y = x + gamma * ( GELU( LN(dwconv7x7(x) + dw_b) @ w1 + b1 ) @ w2 + b2 )
with LN over channels, exact (erf) GELU, NCHW in/out.

Distribution: batch 16 -> 2 images per core across 8 cores. No collectives.

Everything heavy runs on the tensor engine in fp8 DoubleRow mode (2 fp8
k-blocks per pass, 0.5 cycles/row):
 - depthwise 7x7 conv: channels on partitions, diagonal-weight matmuls over a
   zero-padded 62x62 fp8 image; 49 taps + 1 zero tap = 25 DR pairs per
   448-pixel chunk, moving APs [128, 2, 8, 56] built directly with pair
   stride = tap-offset delta (weights pre-scaled x256, undone in the
   psum->acc bias activation).
 - MLP d->4d->4d->d: fp8 DR over k-blocks; the LN is folded in: xq holds
   y*rstd in blocks 0-2 and a 4th block whose row0 = mu*rstd (mean
   correction via s1n row) and row1 = 1.0 (carries b1), both x16; w2 carries
   x32, b2 rides a k=1 matmul of 32*b2 against a const-ones row; gamma/32 and
   the fp32 residual are applied by one DVE scalar_tensor_tensor straight
   from PSUM.
LN stats: ones-matmuls for mean/meansq (bf16), variance -> DRAM-bounced
[56,8] transposed layout, magic-constant Newton rsqrt on DVE (2 iters),
rstd broadcast to 128 partitions by DMA. Squares batched on ACT.
"""

import sys

sys.path.insert(0, "/opt/trn_rl_repo")

import numpy as np
import ml_dtypes

import bass_rust
import concourse.bass as bass
import concourse.mybir as mybir
import concourse.tile as tile
from concourse.bass_utils import run_bass_kernel_spmd

F32 = mybir.dt.float32
BF16 = mybir.dt.bfloat16
FP8 = mybir.dt.float8e4
I32 = mybir.dt.int32
AF = mybir.ActivationFunctionType
ALU = mybir.AluOpType
DRM = mybir.MatmulPerfMode.DoubleRow

N_CORES = 8
IMGS_PER_CORE = 2
C = 384
CB = 3          # channel blocks of 128
H = W = 56
PIX = H * W     # 3136
RPAD = 62       # 3 + 56 + 3 rows
WPAD = 62       # 3 + 56 + 3 cols
CHUNK = 448     # pixels per chunk (8 rows)
NCHUNK = 7
FD = 1536       # hidden dim
NFC = 12        # hidden blocks of 128
EPS = 1e-6

WSCALE = 256.0   # conv tap weights
W1SCALE = 16.0   # w1 / s1n / b1 rows
W2SCALE = 32.0   # w2 / b2 row

MAGIC = 0x5F3759DF

_WAITSPLIT_N = [0]


def _split_waits(nc, max_waits=1):
    """This walrus build rejects instructions with more than one sync-wait
    command; hoist excess waits onto dedicated NoOps on the same engine."""
    for fn in nc.m.functions:
        for bb in fn.blocks:
            insts = bb.instructions
            idx = 0
            while idx < len(insts):
                ins = insts[idx]
                si = ins.sync_info
                if si is not None and len(si.on_wait) > max_waits:
                    waits = list(si.on_wait)
                    extra, keep = waits[:-max_waits], waits[-max_waits:]
                    nops = []
                    for w in extra:
                        _WAITSPLIT_N[0] += 1
                        nops.append(
                            mybir.InstNoOp(
                                name=f"I-wsplit-{_WAITSPLIT_N[0]}",
                                engine=ins.engine,
                                ins=[],
                                outs=[],
                                sync_info=bass_rust.SyncInfo(
                                    on_wait=[w], on_update=[]
                                ),
                            )
                        )
                    ins.sync_info = bass_rust.SyncInfo(
                        on_wait=keep, on_update=list(si.on_update)
                    )
                    insts[idx:idx] = nops
                    idx += len(nops)
                idx += 1


def _sv(base_ap, extra_off, dims):
    """Strided AP view over base_ap's tensor: partition dim kept, free dims
    given as (size, stride) pairs, offset in elements."""
    pairs = [list(base_ap.ap[0])] + [[s, n] for n, s in dims]
    return bass.AP(base_ap.tensor, base_ap.offset + extra_off,
                   bass_rust.VecI64Pair(pairs))


def _de_off(de, hc0):
    d, e = de
    return (hc0 + d + 3) * WPAD + (e + 3)


def _conv_pairs():
    """49 taps + 1 zero slot as 25 DoubleRow pairs. The pair-dim stride of
    the moving AP must be EVEN (odd fp8 strides hang the PE): row pairs
    (d,e)&(d+1,e) have delta 62; row-3 pairs (3,e)&(3,e+2) have delta 2;
    the leftover (3,2) rides second in a pair whose first slot is a zero
    tap reading 2 elements earlier."""
    pairs = []
    for d in (-3, -1, 1):
        for e in range(-3, 4):
            pairs.append(((d, e), (d + 1, e)))
    for e1, e2 in ((-3, -1), (-2, 0), (1, 3)):
        pairs.append(((3, e1), (3, e2)))
    pairs.append((None, (3, 2)))
    return pairs


PAIRS = _conv_pairs()


def _build_nc():
    nc = bass.Bass(trn_type="TRN2", target_bir_lowering=False, debug=False)

    xs = nc.dram_tensor("xs", [IMGS_PER_CORE, C, H, W], F32, kind="ExternalInput")
    dg = nc.dram_tensor("dg", [128, CB, 50, 128], FP8, kind="ExternalInput")
    dwb = nc.dram_tensor("dwb", [C], F32, kind="ExternalInput")
    w1q = nc.dram_tensor("w1q", [128, 4, FD], FP8, kind="ExternalInput")
    w2q = nc.dram_tensor("w2q", [128, NFC, C], FP8, kind="ExternalInput")
    b1p = nc.dram_tensor("b1p", [FD], F32, kind="ExternalInput")
    b2r = nc.dram_tensor("b2r", [1, C], FP8, kind="ExternalInput")
    gsc = nc.dram_tensor("gsc", [C], F32, kind="ExternalInput")
    ys = nc.dram_tensor("ys", [IMGS_PER_CORE, C, H, W], F32, kind="ExternalOutput")
    vscratch = nc.dram_tensor("vscratch", [IMGS_PER_CORE, PIX], F32, kind="Internal")
    rscratch = nc.dram_tensor("rscratch", [IMGS_PER_CORE, PIX], F32, kind="Internal")

    xs3 = xs.ap().rearrange("i c h w -> i c (h w)")
    ys3 = ys.ap().rearrange("i c h w -> i c (h w)")

    with tile.TileContext(nc) as tc:
        with (
            tc.tile_pool(name="const", bufs=1) as constp,
            tc.tile_pool(name="xstage", bufs=2) as xstagep,
            tc.tile_pool(name="xpad", bufs=1) as xpadp,
            tc.tile_pool(name="acc", bufs=1) as accp,
            tc.tile_pool(name="ysq", bufs=2) as ysqp,
            tc.tile_pool(name="xq", bufs=2) as xqp,
            tc.tile_pool(name="hq", bufs=2) as hqp,
            tc.tile_pool(name="rb", bufs=2) as rbp,
            tc.tile_pool(name="small", bufs=3) as smallp,
            tc.tile_pool(name="out", bufs=3) as outp,
            tc.tile_pool(name="psconv", bufs=2, space="PSUM") as psconvp,
            tc.tile_pool(name="ps1", bufs=2, space="PSUM") as ps1p,
            tc.tile_pool(name="ps2", bufs=2, space="PSUM") as ps2p,
            tc.tile_pool(name="psstat", bufs=2, space="PSUM") as psstatp,
        ):
            # ---- static weights / constants ----
            dg_sb = constp.tile([128, CB, 50, 128], FP8)
            nc.sync.dma_start(dg_sb[:], dg.ap())
            dwb_sb = constp.tile([128, CB], F32)
            nc.sync.dma_start(dwb_sb[:], dwb.ap().rearrange("(cb p) -> p cb", p=128))
            w1_sb = constp.tile([128, 4, FD], FP8)
            nc.sync.dma_start(w1_sb[:], w1q.ap())
            w2_sb = constp.tile([128, NFC, C], FP8)
            nc.sync.dma_start(w2_sb[:], w2q.ap())
            b1_sb = constp.tile([128, NFC], F32)
            nc.sync.dma_start(b1_sb[:], b1p.ap().rearrange("(fc p) -> p fc", p=128))
            b2_sb = constp.tile([1, C], FP8)
            nc.sync.dma_start(b2_sb[:], b2r.ap())
            gsc_sb = constp.tile([128, CB], F32)
            nc.sync.dma_start(gsc_sb[:], gsc.ap().rearrange("(cb p) -> p cb", p=128))
            ones_bf = constp.tile([128, 1], BF16)
            nc.gpsimd.memset(ones_bf[:], 1.0)
            ones8 = constp.tile([1, CHUNK], FP8)
            nc.gpsimd.memset(ones8[:], 1.0)
            s256 = constp.tile([128, 1], F32)
            nc.gpsimd.memset(s256[:], 1.0 / WSCALE)
            s16 = constp.tile([128, 1], F32)
            nc.gpsimd.memset(s16[:], 1.0 / W1SCALE)

            # padded fp8 image tiles, one per (img, cb); pads zeroed in the
            # prologue below, ordered so the first conv inputs land early
            xpads = {}
            for img in range(IMGS_PER_CORE):
                for cb in range(CB):
                    t = xpadp.tile([128, RPAD * WPAD], FP8, tag=f"xpad{img}{cb}", name=f"xpad{img}{cb}")
                    xpads[(img, cb)] = t

            accs = {}
            for img in range(IMGS_PER_CORE):
                acc_t = accp.tile([128, CB, PIX], BF16, tag=f"acc{img}", name=f"acc{img}")
                accs[img] = acc_t

            # ---------------- emission helpers ----------------
            def emit_xstage_dma(img, cb):
                cs = slice(cb * 128, (cb + 1) * 128)
                xstg = xstagep.tile([128, PIX], BF16, tag="xstg", name="xstg")
                nc.gpsimd.dma_start(xstg[:], xs3[img, cs])
                return xstg

            def emit_input_prep(img, cb, xstg=None):
                if xstg is None:
                    xstg = emit_xstage_dma(img, cb)
                xp = xpads[(img, cb)]
                xp3 = xp.rearrange("p (r w) -> p r w", w=WPAD)
                nc.gpsimd.tensor_copy(
                    xp3[:, 3:59, 3:59],
                    xstg.rearrange("p (h w) -> p h w", w=W),
                )

            def emit_conv_chunk(img, ch):
                hc0 = ch * 8
                for cb in range(CB):
                    xp = xpads[(img, cb)]
                    pst = psconvp.tile([128, CHUNK], F32, tag="psconv")
                    for p, (t1, t2) in enumerate(PAIRS):
                        if t1 is None:
                            base = _de_off(t2, hc0) - 2
                            delta = 2
                        else:
                            base = _de_off(t1, hc0)
                            delta = _de_off(t2, hc0) - base
                        nc.tensor.matmul(
                            pst[:], dg_sb[:, cb, 2 * p : 2 * p + 2, :],
                            _sv(xp[:], base, [(2, delta), (8, WPAD), (56, 1)]),
                            start=(p == 0), stop=(p == len(PAIRS) - 1),
                            perf_mode=DRM,
                        )
                    nc.scalar.activation(
                        accs[img][:, cb, hc0 * W : hc0 * W + CHUNK], pst[:],
                        AF.Identity, bias=dwb_sb[:, cb : cb + 1],
                        scale=s256[:, 0:1],
                    )

            def emit_stats(img, ch):
                """LN stats + rstd chain + xq for one chunk. Returns xq tile."""
                sl = slice(ch * CHUNK, (ch + 1) * CHUNK)
                acc = accs[img]
                pmu = psstatp.tile([1, CHUNK], F32, tag="ps_stat")
                for cb in range(CB):
                    nc.tensor.matmul(
                        pmu[:], ones_bf[:, 0:1], acc[:, cb, sl],
                        start=(cb == 0), stop=(cb == CB - 1),
                    )
                mu32 = smallp.tile([1, CHUNK], F32, tag="mu32")
                nc.vector.tensor_scalar_mul(mu32[:], pmu[:], 1.0 / C)
                ysq = ysqp.tile([128, CB, CHUNK], BF16, tag="ysq")
                nc.scalar.activation(ysq[:], acc[:, :, sl], AF.Square)
                pmsq = psstatp.tile([1, CHUNK], F32, tag="ps_stat")
                for cb in range(CB):
                    nc.tensor.matmul(
                        pmsq[:], ones_bf[:, 0:1], ysq[:, cb, :],
                        start=(cb == 0), stop=(cb == CB - 1),
                    )
                tq = smallp.tile([1, CHUNK], F32, tag="tq")
                nc.vector.tensor_mul(tq[:], mu32[:], mu32[:])
                vchunk = smallp.tile([1, CHUNK], F32, tag="vchunk")
                nc.vector.scalar_tensor_tensor(
                    out=vchunk[:], in0=pmsq[:], scalar=1.0 / C, in1=tq[:],
                    op0=ALU.mult, op1=ALU.subtract,
                )
                nc.sync.dma_start(vscratch.ap()[img : img + 1, sl], vchunk[0:1, :])

                # Newton rsqrt in [56,8] transposed layout (DRAM bounce)
                vpf = smallp.tile([56, 8], F32, tag="vpf")
                nc.sync.dma_start(
                    vpf[:], vscratch.ap()[img, sl].rearrange("(p f) -> p f", p=56)
                )
                v_eps = smallp.tile([56, 8], F32, tag="veps")
                nc.vector.tensor_scalar_add(v_eps[:], vpf[:], EPS)
                yr = smallp.tile([56, 8], F32, tag="yr")
                ti = smallp.tile([56, 8], I32, tag="ti")
                nc.vector.tensor_scalar(
                    ti[:], v_eps[:].bitcast(I32), 1, None, ALU.logical_shift_right
                )
                nc.vector.tensor_scalar(ti[:], ti[:], -1, None, ALU.bitwise_xor)
                nc.vector.tensor_scalar(yr[:].bitcast(I32), ti[:], MAGIC + 1, None, ALU.add)
                rr = smallp.tile([56, 8], F32, tag="rr")
                for _ in range(2):
                    nc.vector.tensor_mul(rr[:], yr[:], yr[:])
                    nc.vector.tensor_mul(rr[:], rr[:], v_eps[:])
                    nc.vector.tensor_scalar(rr[:], rr[:], -0.5, 1.5, ALU.mult, ALU.add)
                    nc.vector.tensor_mul(yr[:], yr[:], rr[:])
                nc.sync.dma_start(
                    rscratch.ap()[img, sl].rearrange("(p f) -> p f", p=56), yr[:]
                )
                rb = rbp.tile([128, CHUNK], BF16, tag="rb")
                nc.gpsimd.dma_start(rb[:], rscratch.ap()[img, sl].partition_broadcast(128))

                xq = xqp.tile([128, 4, CHUNK], FP8, tag="xq")
                nc.gpsimd.memset(xq[:, 3, :], 0.0)
                nc.vector.tensor_mul(xq[0:1, 3, :], mu32[:], rb[0:1, :])
                for cb in range(CB):
                    nc.vector.tensor_mul(xq[:, cb, :], acc[:, cb, sl], rb[:])
                return xq

            def emit_p1(img, ch, xq):
                hq = hqp.tile([128, NFC, CHUNK], FP8, tag="hq")
                for fc in range(NFC):
                    fsl = slice(fc * 128, (fc + 1) * 128)
                    p1 = ps1p.tile([128, CHUNK], F32, tag="p1")
                    nc.tensor.matmul(p1[:], w1_sb[:, 0:2, fsl], xq[:, 0:2, :],
                                     start=True, stop=False, perf_mode=DRM)
                    nc.tensor.matmul(p1[:], w1_sb[:, 2:4, fsl], xq[:, 2:4, :],
                                     start=False, stop=True, perf_mode=DRM)
                    nc.scalar.activation(hq[:, fc, :], p1[:], AF.Gelu,
                                         bias=b1_sb[:, fc : fc + 1],
                                         scale=s16[:, 0:1])
                return hq

            def emit_p2(img, ch, hq):
                sl = slice(ch * CHUNK, (ch + 1) * CHUNK)
                for cb in range(CB):
                    cs = slice(cb * 128, (cb + 1) * 128)
                    p2 = ps2p.tile([128, CHUNK], F32, tag="p2")
                    for j in range(NFC // 2):
                        nc.tensor.matmul(
                            p2[:], w2_sb[:, 2 * j : 2 * j + 2, cs],
                            hq[:, 2 * j : 2 * j + 2, :],
                            start=(j == 0), stop=False, perf_mode=DRM,
                        )
                    nc.tensor.matmul(p2[:], b2_sb[:, cs], ones8[:, :],
                                     start=False, stop=True)
                    xres = outp.tile([128, CHUNK], F32, tag="xres")
                    nc.sync.dma_start(xres[:], xs3[img, cs, sl])
                    ysb = outp.tile([128, CHUNK], F32, tag="ysb")
                    nc.vector.scalar_tensor_tensor(
                        out=ysb[:], in0=p2[:], scalar=gsc_sb[:, cb : cb + 1],
                        in1=xres[:], op0=ALU.mult, op1=ALU.add,
                    )
                    nc.sync.dma_start(ys3[img, cs, sl], ysb[:])

            # ---------------- software-pipelined emission ----------------
            # PE stream per stage i: stats(i) | conv(i+2) | p1(i) | p2(i-1).
            # conv between stats and p1 hides the rstd chain latency; p2 runs
            # one stage late so ACT's 12 GELUs have a full stage to complete.
            chunks = [(img, ch) for img in range(IMGS_PER_CORE)
                      for ch in range(NCHUNK)]
            # prologue: input DMAs issue before the bulk of the memsets so
            # the first conv chunk's fp8 image is ready ~12us in
            stg00 = emit_xstage_dma(0, 0)
            nc.gpsimd.memset(xpads[(0, 0)][:], 0.0)
            stg01 = emit_xstage_dma(0, 1)
            nc.gpsimd.memset(xpads[(0, 1)][:], 0.0)
            emit_input_prep(0, 0, stg00)
            nc.gpsimd.memset(xpads[(0, 2)][:], 0.0)
            stg02 = emit_xstage_dma(0, 2)
            emit_input_prep(0, 1, stg01)
            emit_input_prep(0, 2, stg02)
            for cb in range(CB):
                nc.gpsimd.memset(xpads[(1, cb)][:], 0.0)
            emit_conv_chunk(*chunks[0])
            emit_conv_chunk(*chunks[1])
            conv_next = 2
            pending = None  # (img, ch, hq) awaiting p2
            for i, (img, ch) in enumerate(chunks):
                if 2 <= i <= 4:
                    emit_input_prep(1, i - 2)
                xq = emit_stats(img, ch)
                if conv_next < len(chunks):
                    emit_conv_chunk(*chunks[conv_next])
                    conv_next += 1
                hq = emit_p1(img, ch, xq)
                if pending is not None:
                    emit_p2(*pending)
                pending = (img, ch, hq)
            emit_p2(*pending)

    _split_waits(nc)
    return nc


_NC_CACHE = None


def _host_fold(inputs):
    dw_w = np.asarray(inputs["dw_w"], dtype=np.float32)
    dw_b = np.asarray(inputs["dw_b"], dtype=np.float32)
    ln_w = np.asarray(inputs["ln_w"], dtype=np.float32)
    ln_b = np.asarray(inputs["ln_b"], dtype=np.float32)
    w1 = np.asarray(inputs["w1"], dtype=np.float32)
    b1 = np.asarray(inputs["b1"], dtype=np.float32)
    w2 = np.asarray(inputs["w2"], dtype=np.float32)
    b2 = np.asarray(inputs["b2"], dtype=np.float32)
    gamma = np.asarray(inputs["gamma"], dtype=np.float32)

    f8 = ml_dtypes.float8_e4m3

    # conv taps -> diag fp8 [128, cb, 50, 128]; slots (2p, 2p+1) hold the
    # two taps of PAIRS[p] (zeros for the dummy slot)
    wt = dw_w[:, :, 0, :] * WSCALE  # [7, 7, C]
    dgq = np.zeros((128, CB, 50, 128), dtype=f8)
    idx = np.arange(128)
    for cb in range(CB):
        for p, pair in enumerate(PAIRS):
            for s, de in enumerate(pair):
                if de is None:
                    continue
                d, e = de
                wq = (wt[d + 3, e + 3, cb * 128 : (cb + 1) * 128]).astype(f8)
                dgq[idx, cb, 2 * p + s, idx] = wq

    w1p = ln_w[:, None] * w1                       # LN scale into w1
    b1p = b1 + ln_b @ w1                           # LN shift into b1
    s1n = -w1p.sum(axis=0)                         # mean-correction row
    w1qa = np.zeros((128, 4, FD), dtype=f8)
    for cb in range(CB):
        w1qa[:, cb, :] = (w1p[cb * 128 : (cb + 1) * 128] * W1SCALE).astype(f8)
    w1qa[0, 3, :] = (s1n * W1SCALE).astype(f8)

    w2qa = np.zeros((128, NFC, C), dtype=f8)
    for j in range(NFC):
        w2qa[:, j, :] = (w2[j * 128 : (j + 1) * 128] * W2SCALE).astype(f8)
    b2q = (b2 * W2SCALE).astype(f8).reshape(1, C)
    gscv = (gamma / W2SCALE).astype(np.float32)

    return {
        "dg": dgq,
        "dwb": dw_b.astype(np.float32),
        "b1p": b1p.astype(np.float32),
        "w1q": w1qa,
        "w2q": w2qa,
        "b2r": b2q,
        "gsc": gscv,
    }


def make_in_maps(inputs):
    x = np.asarray(inputs["x"], dtype=np.float32)
    common = _host_fold(inputs)
    in_maps = []
    for k in range(N_CORES):
        m = dict(common)
        m["xs"] = np.ascontiguousarray(x[k * IMGS_PER_CORE : (k + 1) * IMGS_PER_CORE])
        in_maps.append(m)
    return in_maps


def kernel(**inputs):
    global _NC_CACHE
    in_maps = make_in_maps(inputs)
    if _NC_CACHE is None:
        _NC_CACHE = _build_nc()
    res = run_bass_kernel_spmd(_NC_CACHE, in_maps, core_ids=list(range(N_CORES)))
    out = np.concatenate([res.results[k]["ys"] for k in range(N_CORES)], axis=0)
    return out.astype(np.float32)


if __name__ == "__main__":
    rng = np.random.default_rng(0)
    ins = {
        "x": rng.standard_normal((16, C, H, W), dtype=np.float32),
        "dw_w": 0.02 * rng.standard_normal((7, 7, 1, C), dtype=np.float32),
        "dw_b": 0.02 * rng.standard_normal((C,), dtype=np.float32),
        "ln_w": np.ones(C, np.float32),
        "ln_b": np.zeros(C, np.float32),
        "w1": (C**-0.5) * rng.standard_normal((C, FD), dtype=np.float32),
        "b1": 0.02 * rng.standard_normal((FD,), dtype=np.float32),
        "w2": ((4 * C) ** -0.5) * rng.standard_normal((FD, C), dtype=np.float32),
        "b2": 0.02 * rng.standard_normal((C,), dtype=np.float32),
        "gamma": np.full((C,), 1e-6, np.float32),
    }
    out = kernel(**ins)
    print("out", out.shape, out.dtype, np.abs(out).mean())


# revision 7
# speedup vs baseline: 1.3632x; 1.0277x over previous
"""ConvNeXt block kernel for Trainium2 — fp8 DoubleRow rewrite.

Reference semantics (per image):
  y = x + gamma * ( GELU( LN(dwconv7x7(x) + dw_b) @ w1 + b1 ) @ w2 + b2 )
with LN over channels, exact (erf) GELU, NCHW in/out.

Distribution: batch 16 -> 2 images per core across 8 cores. No collectives.

Everything heavy runs on the tensor engine in fp8 DoubleRow mode (2 fp8
k-blocks per pass, 0.5 cycles/row):
 - depthwise 7x7 conv: channels on partitions, diagonal-weight matmuls over a
   zero-padded 62x62 fp8 image; 49 taps + 1 zero tap = 25 DR pairs per
   448-pixel chunk, moving APs [128, 2, 8, 56] built directly with pair
   stride = tap-offset delta (weights pre-scaled x256, undone in the
   psum->acc bias activation).
 - MLP d->4d->4d->d: fp8 DR over k-blocks; the LN is folded in: xq holds
   y*rstd in blocks 0-2 and a 4th block whose row0 = mu*rstd (mean
   correction via s1n row) and row1 = 1.0 (carries b1), both x16; w2 carries
   x32, b2 rides a k=1 matmul of 32*b2 against a const-ones row; gamma/32 and
   the fp32 residual are applied by one DVE scalar_tensor_tensor straight
   from PSUM.
LN stats: ones-matmuls for mean/meansq (bf16), variance -> DRAM-bounced
[56,8] transposed layout, magic-constant Newton rsqrt on DVE (2 iters),
rstd broadcast to 128 partitions by DMA. Squares batched on ACT.
"""

import sys

sys.path.insert(0, "/opt/trn_rl_repo")

import numpy as np
import ml_dtypes

import bass_rust
import concourse.bass as bass
import concourse.mybir as mybir
import concourse.tile as tile
from concourse.bass_utils import run_bass_kernel_spmd

F32 = mybir.dt.float32
BF16 = mybir.dt.bfloat16
FP8 = mybir.dt.float8e4
I32 = mybir.dt.int32
AF = mybir.ActivationFunctionType
ALU = mybir.AluOpType
DRM = mybir.MatmulPerfMode.DoubleRow

N_CORES = 8
IMGS_PER_CORE = 2
C = 384
CB = 3          # channel blocks of 128
H = W = 56
PIX = H * W     # 3136
RPAD = 62       # 3 + 56 + 3 rows
WPAD = 62       # 3 + 56 + 3 cols
CHUNK = 448     # pixels per chunk (8 rows)
NCHUNK = 7
FD = 1536       # hidden dim
NFC = 12        # hidden blocks of 128
EPS = 1e-6

WSCALE = 256.0   # conv tap weights
W1SCALE = 16.0   # w1 / s1n / b1 rows
W2SCALE = 32.0   # w2 / b2 row

MAGIC = 0x5F3759DF

_WAITSPLIT_N = [0]


def _split_waits(nc, max_waits=1):
    """This walrus build rejects instructions with more than one sync-wait
    command; hoist excess waits onto dedicated NoOps on the same engine."""
    for fn in nc.m.functions:
        for bb in fn.blocks:
            insts = bb.instructions
            idx = 0
            while idx < len(insts):
                ins = insts[idx]
                si = ins.sync_info
                if si is not None and len(si.on_wait) > max_waits:
                    waits = list(si.on_wait)
                    extra, keep = waits[:-max_waits], waits[-max_waits:]
                    nops = []
                    for w in extra:
                        _WAITSPLIT_N[0] += 1
                        nops.append(
                            mybir.InstNoOp(
                                name=f"I-wsplit-{_WAITSPLIT_N[0]}",
                                engine=ins.engine,
                                ins=[],
                                outs=[],
                                sync_info=bass_rust.SyncInfo(
                                    on_wait=[w], on_update=[]
                                ),
                            )
                        )
                    ins.sync_info = bass_rust.SyncInfo(
                        on_wait=keep, on_update=list(si.on_update)
                    )
                    insts[idx:idx] = nops
                    idx += len(nops)
                idx += 1


def _sv(base_ap, extra_off, dims):
    """Strided AP view over base_ap's tensor: partition dim kept, free dims
    given as (size, stride) pairs, offset in elements."""
    pairs = [list(base_ap.ap[0])] + [[s, n] for n, s in dims]
    return bass.AP(base_ap.tensor, base_ap.offset + extra_off,
                   bass_rust.VecI64Pair(pairs))


def _de_off(de, hc0):
    d, e = de
    return (hc0 + d + 3) * WPAD + (e + 3)


def _conv_pairs():
    """49 taps + 1 zero slot as 25 DoubleRow pairs. The pair-dim stride of
    the moving AP must be EVEN (odd fp8 strides hang the PE): row pairs
    (d,e)&(d+1,e) have delta 62; row-3 pairs (3,e)&(3,e+2) have delta 2;
    the leftover (3,2) rides second in a pair whose first slot is a zero
    tap reading 2 elements earlier."""
    pairs = []
    for d in (-3, -1, 1):
        for e in range(-3, 4):
            pairs.append(((d, e), (d + 1, e)))
    for e1, e2 in ((-3, -1), (-2, 0), (1, 3)):
        pairs.append(((3, e1), (3, e2)))
    pairs.append((None, (3, 2)))
    return pairs


PAIRS = _conv_pairs()


def _build_nc():
    nc = bass.Bass(trn_type="TRN2", target_bir_lowering=False, debug=False)

    xs = nc.dram_tensor("xs", [IMGS_PER_CORE, C, H, W], F32, kind="ExternalInput")
    dg = nc.dram_tensor("dg", [128, CB, 50, 128], FP8, kind="ExternalInput")
    dwb = nc.dram_tensor("dwb", [C], F32, kind="ExternalInput")
    w1q = nc.dram_tensor("w1q", [128, 4, FD], FP8, kind="ExternalInput")
    w2q = nc.dram_tensor("w2q", [128, NFC + 2, C], FP8, kind="ExternalInput")
    b1p = nc.dram_tensor("b1p", [FD], F32, kind="ExternalInput")
    gsc = nc.dram_tensor("gsc", [C], F32, kind="ExternalInput")
    ys = nc.dram_tensor("ys", [IMGS_PER_CORE, C, H, W], F32, kind="ExternalOutput")
    vscratch = nc.dram_tensor("vscratch", [IMGS_PER_CORE, PIX], F32, kind="Internal")
    rscratch = nc.dram_tensor("rscratch", [IMGS_PER_CORE, PIX], F32, kind="Internal")

    xs3 = xs.ap().rearrange("i c h w -> i c (h w)")
    ys3 = ys.ap().rearrange("i c h w -> i c (h w)")

    with tile.TileContext(nc) as tc:
        with (
            tc.tile_pool(name="const", bufs=1) as constp,
            tc.tile_pool(name="xstage", bufs=2) as xstagep,
            tc.tile_pool(name="xpad", bufs=1) as xpadp,
            tc.tile_pool(name="acc", bufs=1) as accp,
            tc.tile_pool(name="ysq", bufs=2) as ysqp,
            tc.tile_pool(name="xq", bufs=2) as xqp,
            tc.tile_pool(name="hq", bufs=2) as hqp,
            tc.tile_pool(name="rb", bufs=2) as rbp,
            tc.tile_pool(name="small", bufs=3) as smallp,
            tc.tile_pool(name="out", bufs=3) as outp,
            tc.tile_pool(name="psconv", bufs=2, space="PSUM") as psconvp,
            tc.tile_pool(name="ps1", bufs=2, space="PSUM") as ps1p,
            tc.tile_pool(name="ps2", bufs=2, space="PSUM") as ps2p,
            tc.tile_pool(name="psstat", bufs=2, space="PSUM") as psstatp,
        ):
            # ---- static weights / constants ----
            dg_sb = constp.tile([128, CB, 50, 128], FP8)
            nc.sync.dma_start(dg_sb[:], dg.ap())
            dwb_sb = constp.tile([128, CB], F32)
            nc.sync.dma_start(dwb_sb[:], dwb.ap().rearrange("(cb p) -> p cb", p=128))
            w1_sb = constp.tile([128, 4, FD], FP8)
            nc.sync.dma_start(w1_sb[:], w1q.ap())
            w2_sb = constp.tile([128, NFC + 2, C], FP8)
            nc.sync.dma_start(w2_sb[:], w2q.ap())
            b1_sb = constp.tile([128, NFC], F32)
            nc.sync.dma_start(b1_sb[:], b1p.ap().rearrange("(fc p) -> p fc", p=128))
            gsc_sb = constp.tile([128, CB], F32)
            nc.sync.dma_start(gsc_sb[:], gsc.ap().rearrange("(cb p) -> p cb", p=128))
            ones_bf = constp.tile([128, 1], BF16)
            nc.gpsimd.memset(ones_bf[:], 1.0)
            s256 = constp.tile([128, 1], F32)
            nc.gpsimd.memset(s256[:], 1.0 / WSCALE)
            s16 = constp.tile([128, 1], F32)
            nc.gpsimd.memset(s16[:], 1.0 / W1SCALE)

            # padded fp8 image tiles, one per (img, cb); pads zeroed in the
            # prologue below, ordered so the first conv inputs land early
            xpads = {}
            for img in range(IMGS_PER_CORE):
                for cb in range(CB):
                    t = xpadp.tile([128, RPAD * WPAD], FP8, tag=f"xpad{img}{cb}", name=f"xpad{img}{cb}")
                    xpads[(img, cb)] = t

            accs = {}
            for img in range(IMGS_PER_CORE):
                acc_t = accp.tile([128, CB, PIX], BF16, tag=f"acc{img}", name=f"acc{img}")
                accs[img] = acc_t

            # ---------------- emission helpers ----------------
            def emit_xstage_dma(img, cb):
                cs = slice(cb * 128, (cb + 1) * 128)
                xstg = xstagep.tile([128, PIX], BF16, tag="xstg", name="xstg")
                nc.gpsimd.dma_start(xstg[:], xs3[img, cs])
                return xstg

            def emit_input_prep(img, cb, xstg=None):
                if xstg is None:
                    xstg = emit_xstage_dma(img, cb)
                xp = xpads[(img, cb)]
                xp3 = xp.rearrange("p (r w) -> p r w", w=WPAD)
                nc.gpsimd.tensor_copy(
                    xp3[:, 3:59, 3:59],
                    xstg.rearrange("p (h w) -> p h w", w=W),
                )

            def emit_conv_chunk(img, ch):
                hc0 = ch * 8
                for cb in range(CB):
                    xp = xpads[(img, cb)]
                    pst = psconvp.tile([128, CHUNK], F32, tag="psconv")
                    for p, (t1, t2) in enumerate(PAIRS):
                        if t1 is None:
                            base = _de_off(t2, hc0) - 2
                            delta = 2
                        else:
                            base = _de_off(t1, hc0)
                            delta = _de_off(t2, hc0) - base
                        nc.tensor.matmul(
                            pst[:], dg_sb[:, cb, 2 * p : 2 * p + 2, :],
                            _sv(xp[:], base, [(2, delta), (8, WPAD), (56, 1)]),
                            start=(p == 0), stop=(p == len(PAIRS) - 1),
                            perf_mode=DRM,
                        )
                    nc.scalar.activation(
                        accs[img][:, cb, hc0 * W : hc0 * W + CHUNK], pst[:],
                        AF.Identity, bias=dwb_sb[:, cb : cb + 1],
                        scale=s256[:, 0:1],
                    )

            def emit_stats(img, ch):
                """LN stats + rstd chain + xq for one chunk. Returns xq tile."""
                sl = slice(ch * CHUNK, (ch + 1) * CHUNK)
                acc = accs[img]
                pmu = psstatp.tile([1, CHUNK], F32, tag="ps_stat")
                for cb in range(CB):
                    nc.tensor.matmul(
                        pmu[:], ones_bf[:, 0:1], acc[:, cb, sl],
                        start=(cb == 0), stop=(cb == CB - 1),
                    )
                mu32 = smallp.tile([1, CHUNK], F32, tag="mu32")
                nc.vector.tensor_scalar_mul(mu32[:], pmu[:], 1.0 / C)
                ysq = ysqp.tile([128, CB, CHUNK], BF16, tag="ysq")
                nc.scalar.activation(ysq[:], acc[:, :, sl], AF.Square)
                pmsq = psstatp.tile([1, CHUNK], F32, tag="ps_stat")
                for cb in range(CB):
                    nc.tensor.matmul(
                        pmsq[:], ones_bf[:, 0:1], ysq[:, cb, :],
                        start=(cb == 0), stop=(cb == CB - 1),
                    )
                tq = smallp.tile([1, CHUNK], F32, tag="tq")
                nc.vector.tensor_mul(tq[:], mu32[:], mu32[:])
                vchunk = smallp.tile([1, CHUNK], F32, tag="vchunk")
                nc.vector.scalar_tensor_tensor(
                    out=vchunk[:], in0=pmsq[:], scalar=1.0 / C, in1=tq[:],
                    op0=ALU.mult, op1=ALU.subtract,
                )
                nc.sync.dma_start(vscratch.ap()[img : img + 1, sl], vchunk[0:1, :])

                # Newton rsqrt in [56,8] transposed layout (DRAM bounce)
                vpf = smallp.tile([56, 8], F32, tag="vpf")
                nc.sync.dma_start(
                    vpf[:], vscratch.ap()[img, sl].rearrange("(p f) -> p f", p=56)
                )
                v_eps = smallp.tile([56, 8], F32, tag="veps")
                nc.vector.tensor_scalar_add(v_eps[:], vpf[:], EPS)
                yr = smallp.tile([56, 8], F32, tag="yr")
                ti = smallp.tile([56, 8], I32, tag="ti")
                nc.vector.tensor_scalar(
                    ti[:], v_eps[:].bitcast(I32), 1, None, ALU.logical_shift_right
                )
                nc.vector.tensor_scalar(ti[:], ti[:], -1, None, ALU.bitwise_xor)
                nc.vector.tensor_scalar(yr[:].bitcast(I32), ti[:], MAGIC + 1, None, ALU.add)
                rr = smallp.tile([56, 8], F32, tag="rr")
                for _ in range(2):
                    nc.vector.tensor_mul(rr[:], yr[:], yr[:])
                    nc.vector.tensor_mul(rr[:], rr[:], v_eps[:])
                    nc.vector.tensor_scalar(rr[:], rr[:], -0.5, 1.5, ALU.mult, ALU.add)
                    nc.vector.tensor_mul(yr[:], yr[:], rr[:])
                nc.sync.dma_start(
                    rscratch.ap()[img, sl].rearrange("(p f) -> p f", p=56), yr[:]
                )
                rb = rbp.tile([128, CHUNK], BF16, tag="rb")
                nc.gpsimd.dma_start(rb[:], rscratch.ap()[img, sl].partition_broadcast(128))

                xq = xqp.tile([128, 4, CHUNK], FP8, tag="xq")
                nc.gpsimd.memset(xq[:, 3, :], 0.0)
                nc.vector.tensor_mul(xq[0:1, 3, :], mu32[:], rb[0:1, :])
                for cb in range(CB):
                    nc.vector.tensor_mul(xq[:, cb, :], acc[:, cb, sl], rb[:])
                return xq

            def emit_p1(img, ch, xq):
                hq = hqp.tile([128, NFC + 2, CHUNK], FP8, tag="hq")
                nc.gpsimd.memset(hq[:, NFC : NFC + 2, :], 0.0)
                nc.gpsimd.memset(hq[0:1, NFC, :], 1.0)
                for fc in range(NFC):
                    fsl = slice(fc * 128, (fc + 1) * 128)
                    p1 = ps1p.tile([128, CHUNK], F32, tag="p1")
                    nc.tensor.matmul(p1[:], w1_sb[:, 0:2, fsl], xq[:, 0:2, :],
                                     start=True, stop=False, perf_mode=DRM)
                    nc.tensor.matmul(p1[:], w1_sb[:, 2:4, fsl], xq[:, 2:4, :],
                                     start=False, stop=True, perf_mode=DRM)
                    nc.scalar.activation(hq[:, fc, :], p1[:], AF.Gelu,
                                         bias=b1_sb[:, fc : fc + 1],
                                         scale=s16[:, 0:1])
                return hq

            def emit_p2(img, ch, hq):
                sl = slice(ch * CHUNK, (ch + 1) * CHUNK)
                for cb in range(CB):
                    cs = slice(cb * 128, (cb + 1) * 128)
                    p2 = ps2p.tile([128, CHUNK], F32, tag="p2")
                    for j in range(NFC // 2 + 1):
                        nc.tensor.matmul(
                            p2[:], w2_sb[:, 2 * j : 2 * j + 2, cs],
                            hq[:, 2 * j : 2 * j + 2, :],
                            start=(j == 0), stop=(j == NFC // 2),
                            perf_mode=DRM,
                        )
                    xres = outp.tile([128, CHUNK], F32, tag="xres")
                    nc.sync.dma_start(xres[:], xs3[img, cs, sl])
                    ysb = outp.tile([128, CHUNK], F32, tag="ysb")
                    nc.vector.scalar_tensor_tensor(
                        out=ysb[:], in0=p2[:], scalar=gsc_sb[:, cb : cb + 1],
                        in1=xres[:], op0=ALU.mult, op1=ALU.add,
                    )
                    nc.sync.dma_start(ys3[img, cs, sl], ysb[:])

            # ---------------- software-pipelined emission ----------------
            # PE stream per stage i: stats(i) | conv(i+2) | p1(i) | p2(i-1).
            # conv between stats and p1 hides the rstd chain latency; p2 runs
            # one stage late so ACT's 12 GELUs have a full stage to complete.
            chunks = [(img, ch) for img in range(IMGS_PER_CORE)
                      for ch in range(NCHUNK)]
            # prologue: input DMAs issue before the bulk of the memsets so
            # the first conv chunk's fp8 image is ready ~12us in
            stg00 = emit_xstage_dma(0, 0)
            nc.gpsimd.memset(xpads[(0, 0)][:], 0.0)
            stg01 = emit_xstage_dma(0, 1)
            nc.gpsimd.memset(xpads[(0, 1)][:], 0.0)
            emit_input_prep(0, 0, stg00)
            nc.gpsimd.memset(xpads[(0, 2)][:], 0.0)
            stg02 = emit_xstage_dma(0, 2)
            emit_input_prep(0, 1, stg01)
            emit_input_prep(0, 2, stg02)
            for cb in range(CB):
                nc.gpsimd.memset(xpads[(1, cb)][:], 0.0)
            emit_conv_chunk(*chunks[0])
            emit_conv_chunk(*chunks[1])
            emit_conv_chunk(*chunks[2])
            conv_next = 3
            pending = None  # (img, ch, hq) awaiting p2
            for i, (img, ch) in enumerate(chunks):
                if 2 <= i <= 4:
                    emit_input_prep(1, i - 2)
                xq = emit_stats(img, ch)
                if conv_next < len(chunks):
                    emit_conv_chunk(*chunks[conv_next])
                    conv_next += 1
                hq = emit_p1(img, ch, xq)
                if pending is not None:
                    emit_p2(*pending)
                pending = (img, ch, hq)
            emit_p2(*pending)

    _split_waits(nc)
    return nc


_NC_CACHE = None


def _host_fold(inputs):
    dw_w = np.asarray(inputs["dw_w"], dtype=np.float32)
    dw_b = np.asarray(inputs["dw_b"], dtype=np.float32)
    ln_w = np.asarray(inputs["ln_w"], dtype=np.float32)
    ln_b = np.asarray(inputs["ln_b"], dtype=np.float32)
    w1 = np.asarray(inputs["w1"], dtype=np.float32)
    b1 = np.asarray(inputs["b1"], dtype=np.float32)
    w2 = np.asarray(inputs["w2"], dtype=np.float32)
    b2 = np.asarray(inputs["b2"], dtype=np.float32)
    gamma = np.asarray(inputs["gamma"], dtype=np.float32)

    f8 = ml_dtypes.float8_e4m3

    # conv taps -> diag fp8 [128, cb, 50, 128]; slots (2p, 2p+1) hold the
    # two taps of PAIRS[p] (zeros for the dummy slot)
    wt = dw_w[:, :, 0, :] * WSCALE  # [7, 7, C]
    dgq = np.zeros((128, CB, 50, 128), dtype=f8)
    idx = np.arange(128)
    for cb in range(CB):
        for p, pair in enumerate(PAIRS):
            for s, de in enumerate(pair):
                if de is None:
                    continue
                d, e = de
                wq = (wt[d + 3, e + 3, cb * 128 : (cb + 1) * 128]).astype(f8)
                dgq[idx, cb, 2 * p + s, idx] = wq

    w1p = ln_w[:, None] * w1                       # LN scale into w1
    b1p = b1 + ln_b @ w1                           # LN shift into b1
    s1n = -w1p.sum(axis=0)                         # mean-correction row
    w1qa = np.zeros((128, 4, FD), dtype=f8)
    for cb in range(CB):
        w1qa[:, cb, :] = (w1p[cb * 128 : (cb + 1) * 128] * W1SCALE).astype(f8)
    w1qa[0, 3, :] = (s1n * W1SCALE).astype(f8)

    w2qa = np.zeros((128, NFC + 2, C), dtype=f8)
    for j in range(NFC):
        w2qa[:, j, :] = (w2[j * 128 : (j + 1) * 128] * W2SCALE).astype(f8)
    w2qa[0, NFC, :] = (b2 * W2SCALE).astype(f8)
    gscv = (gamma / W2SCALE).astype(np.float32)

    return {
        "dg": dgq,
        "dwb": dw_b.astype(np.float32),
        "b1p": b1p.astype(np.float32),
        "w1q": w1qa,
        "w2q": w2qa,
        "gsc": gscv,
    }


def make_in_maps(inputs):
    x = np.asarray(inputs["x"], dtype=np.float32)
    common = _host_fold(inputs)
    in_maps = []
    for k in range(N_CORES):
        m = dict(common)
        m["xs"] = np.ascontiguousarray(x[k * IMGS_PER_CORE : (k + 1) * IMGS_PER_CORE])
        in_maps.append(m)
    return in_maps


def kernel(**inputs):
    global _NC_CACHE
    in_maps = make_in_maps(inputs)
    if _NC_CACHE is None:
        _NC_CACHE = _build_nc()
    res = run_bass_kernel_spmd(_NC_CACHE, in_maps, core_ids=list(range(N_CORES)))
    out = np.concatenate([res.results[k]["ys"] for k in range(N_CORES)], axis=0)
    return out.astype(np.float32)


if __name__ == "__main__":
    rng = np.random.default_rng(0)
    ins = {
        "x": rng.standard_normal((16, C, H, W), dtype=np.float32),
        "dw_w": 0.02 * rng.standard_normal((7, 7, 1, C), dtype=np.float32),
        "dw_b": 0.02 * rng.standard_normal((C,), dtype=np.float32),
        "ln_w": np.ones(C, np.float32),
        "ln_b": np.zeros(C, np.float32),
        "w1": (C**-0.5) * rng.standard_normal((C, FD), dtype=np.float32),
        "b1": 0.02 * rng.standard_normal((FD,), dtype=np.float32),
        "w2": ((4 * C) ** -0.5) * rng.standard_normal((FD, C), dtype=np.float32),
        "b2": 0.02 * rng.standard_normal((C,), dtype=np.float32),
        "gamma": np.full((C,), 1e-6, np.float32),
    }
    out = kernel(**ins)
    print("out", out.shape, out.dtype, np.abs(out).mean())
